# revision 20
# baseline (speedup 1.0000x reference)
"""Causal multi-head attention with RoPE on 8 Trainium2 NeuronCores.

Sharding: core c -> batch b = c//2, head-group g = c%2 (8 of 16 heads).
Each core computes q/k/v projections for its batch+heads (fp32r matmuls),
applies RoPE (evens/odds row-permuted layout so the pair-rotation becomes
a 32-partition-group swap done by SBUF-SBUF DMA), runs flash-style causal
attention with transposed scores (softmax sum via an appended ones column
of V -> denominator row in the AV PSUM tile), and a partial output
projection over its head group. Host sums the two per-batch partials.

v2: single interleaved phase structure (projections of quarter q+1 and
output projection of block j-1 provide ready PE work while attention j's
exp chain runs on the Act engine), per-quarter k/v tiles (no false
cross-quarter deps), diagonal score/exp/AV tiles trimmed to the causal
width, exp+psqk drains on Act / everything else elementwise on DVE,
V and attention weights in bf16, and a per-quarter batched softmax
normalization (denominators collected to [8,512], one reciprocal, one
DRAM-bounce broadcast, one in-place multiply).
"""
import math
import sys

sys.path.insert(0, "/opt/trn_rl_repo")

import numpy as np

import concourse.tile as tile
from concourse import bacc, bass, mybir
from concourse.bass_utils import run_bass_kernel_spmd

NUM_HEADS = 16
B, S, D = 4, 2048, 1024
HPC = 8            # heads per core
DK = 64
HD = HPC * DK      # 512 head dims per core
THETA = 10000.0
N_CORES = 8
KC = D // 128      # 8 contraction chunks for projections
NQ = 4             # s-quarters of 512

f32 = mybir.dt.float32
f32r = mybir.dt.float32r
bf16 = mybir.dt.bfloat16
ActF = mybir.ActivationFunctionType
Mult = mybir.AluOpType.mult

_prog_cache = {}


def _build_program():
    nc = bacc.Bacc("TRN2", target_bir_lowering=False, debug=False,
                   enable_asserts=False, num_devices=N_CORES)
    xT_d = nc.dram_tensor("xT", [D, S], f32r, kind="ExternalInput").ap()
    wqk_d = nc.dram_tensor("wqkT", [D, 2 * HD], f32r, kind="ExternalInput").ap()
    wv_d = nc.dram_tensor("wvT", [D, HD], f32r, kind="ExternalInput").ap()
    wo_d = nc.dram_tensor("woT", [HD, D], bf16, kind="ExternalInput").ap()
    cos_d = nc.dram_tensor("cosT", [128, S], f32, kind="ExternalInput").ap()
    sin_d = nc.dram_tensor("sinT", [128, S], f32, kind="ExternalInput").ap()
    mask_d = nc.dram_tensor("mask128", [128, 128], bf16, kind="ExternalInput").ap()
    out_d = nc.dram_tensor("outT", [D, S], f32, kind="ExternalOutput").ap()

    with tile.TileContext(nc) as tc:
        with tc.tile_pool(name="p1", bufs=1) as p1, \
             tc.tile_pool(name="p2", bufs=2) as p2, \
             tc.tile_pool(name="pss", bufs=2, space="PSUM") as pss, \
             tc.tile_pool(name="pso", bufs=1, space="PSUM") as pso_pool, \
             tc.tile_pool(name="ppr", bufs=2, space="PSUM") as ppr, \
             tc.tile_pool(name="drp", bufs=2, space="DRAM") as drp:

            # ---------------- persistent SBUF ----------------------------
            wv_sb = p1.tile([128, KC, HD], f32r, tag="wv")
            wqk_sb = p1.tile([128, KC, 2 * HD], f32r, tag="wqk")
            wo_sb = p1.tile([128, HD // 128, D], bf16, tag="wo")
            mask_sb = p1.tile([128, 128], bf16, tag="mask")
            krot = [p1.tile([128, 4, 512], f32r, tag=f"krot{q}", name=f"krot{q}")
                    for q in range(NQ)]
            vaug = [p1.tile([128, 4, HPC, DK + 1], bf16, tag=f"vaug{q}",
                            name=f"vaug{q}") for q in range(NQ)]

            # weight/table loads: wv+cos/sin+mask+wo on sync HWDGE,
            # wqk on the scalar HWDGE ring (Act idle at startup)
            nc.sync.dma_start(out=wv_sb[:],
                              in_=wv_d.rearrange("(k p) n -> p k n", p=128))
            nc.scalar.dma_start(out=wqk_sb[:],
                                in_=wqk_d.rearrange("(k p) n -> p k n", p=128))
            nc.sync.dma_start(out=mask_sb[:], in_=mask_d[:])
            for q in range(NQ):
                nc.vector.memset(vaug[q][:, :, :, DK], 1.0)
            denq = p1.tile([128, 1024], f32, tag="denq", name="denq")
            nc.vector.memset(denq[:], 1.0)

            pending_oproj = []

            def emit_oproj(jq, attn_t):
                # output projection for block jq: emitted after quarter
                # jq+1's projections so the shared 'pp' PSUM ring grants
                # slots in a feasible execution order
                slj = slice(jq * 512, (jq + 1) * 512)
                for mo in range(D // 128):
                    pso = ppr.tile([128, 512], f32, tag="pp", name="pso")
                    for kc2 in range(HD // 128):
                        nc.tensor.matmul(pso[:],
                                         wo_sb[:, kc2, mo * 128:(mo + 1) * 128],
                                         attn_t[:, kc2, :],
                                         start=(kc2 == 0),
                                         stop=(kc2 == HD // 128 - 1))
                    ob = p2.tile([128, 512], f32, tag="ob", name="ob")
                    nc.vector.tensor_copy(out=ob[:], in_=pso[:])
                    nc.sync.dma_start(out=out_d[mo * 128:(mo + 1) * 128, slj],
                                      in_=ob[:])

            for q in range(NQ):
                sl = slice(q * 512, (q + 1) * 512)

                # ---------------- projections for quarter q --------------
                cosq = p2.tile([128, 512], f32, tag="cosq", name="cosq")
                nc.sync.dma_start(out=cosq[:], in_=cos_d[:, sl])
                sinq = p2.tile([128, 512], f32, tag="sinq", name="sinq")
                nc.sync.dma_start(out=sinq[:], in_=sin_d[:, sl])
                qrot = p2.tile([128, 4, 512], f32r, tag="qrot", name="qrot")

                xq = p2.tile([128, KC, 512], f32r, tag="xq", name="xq")
                nc.gpsimd.dma_start(
                    out=xq[:],
                    in_=xT_d[:, sl].rearrange("(k p) c -> p k c", p=128))

                # V projection: 4 s-tiles of 128
                for m in range(4):
                    psv = ppr.tile([128, 512], f32, tag="pp", name="psv")
                    for k in range(KC):
                        nc.tensor.matmul(psv[:],
                                         xq[:, k, m * 128:(m + 1) * 128],
                                         wv_sb[:, k, :],
                                         start=(k == 0), stop=(k == KC - 1))
                    nc.vector.tensor_copy(
                        out=vaug[q][:, m, :, 0:DK],
                        in_=psv[:].rearrange("p (h d) -> p h d", h=HPC))

                # QK projection + RoPE: 8 output row-chunks
                for m in range(2 * KC // 2):
                    xy = p2.tile([128, 2, 512], f32, tag="xy", name="xy")
                    pq = ppr.tile([128, 512], f32, tag="pp", name="psqk")
                    for k in range(KC):
                        nc.tensor.matmul(pq[:],
                                         wqk_sb[:, k, m * 128:(m + 1) * 128],
                                         xq[:, k, :],
                                         start=(k == 0), stop=(k == KC - 1))
                    nc.scalar.copy(out=xy[:, 0, :], in_=pq[:])
                    for gq in range(4):
                        a, bb = 32 * gq, 32 * (gq ^ 1)
                        nc.gpsimd.dma_start(out=xy[a:a + 32, 1, :],
                                            in_=xy[bb:bb + 32, 0, :])
                    t1 = p2.tile([128, 512], f32, tag="t1", name="t1")
                    nc.vector.tensor_mul(t1[:], xy[:, 0, :], cosq[:])
                    nc.vector.tensor_mul(xy[:, 1, :], xy[:, 1, :], sinq[:])
                    dest = qrot[:, m, :] if m < 4 else krot[q][:, m - 4, :]
                    nc.vector.tensor_add(dest, t1[:], xy[:, 1, :])

                if q == 1:
                    nc.sync.dma_start(
                        out=wo_sb[:],
                        in_=wo_d.rearrange("(k p) m -> p k m", p=128))
                if pending_oproj:
                    emit_oproj(*pending_oproj.pop())

                # ---------------- attention for block j=q -----------------
                # denominators staged at partition starts {0,32,64,96} x 2
                # column blocks: head h -> [32*(h%4), (h//4)*512]
                attn = p2.tile([128, 4, 512], bf16, tag="attn", name="attn")
                n_i = 4 * q + 4
                m2 = bass.AP(tensor=mask_sb[:].tensor, offset=mask_sb[:].offset,
                             ap=[mask_sb[:].ap[0], [0, 2], mask_sb[:].ap[1]])
                for hp in range(4):
                    ps_oA = pso_pool.tile([DK + 1, 512], f32, tag="psoA",
                                          name="ps_oA")
                    ps_oB = pso_pool.tile([DK + 1, 512], f32, tag="psoB",
                                          name="ps_oB")
                    prev = None
                    for i in range(n_i):
                        t = i - 4 * q
                        w0 = 128 * t if t > 0 else 0
                        qi, ti = i // 4, i % 4
                        ps_s = pss.tile([128, 2, 512], f32, tag="pss",
                                        name="ps_s")
                        nc.tensor.matmul(
                            ps_s[:, 0, w0:512],
                            krot[qi][0:64, hp, ti * 128:(ti + 1) * 128],
                            qrot[0:64, hp, w0:512], start=True, stop=True)
                        nc.tensor.matmul(
                            ps_s[:, 1, w0:512],
                            krot[qi][64:128, hp, ti * 128:(ti + 1) * 128],
                            qrot[64:128, hp, w0:512], start=True, stop=True)
                        ex = p2.tile([128, 2, 512], bf16, tag="ex", name="ex")
                        nc.scalar.activation(out=ex[:, :, w0:512],
                                             in_=ps_s[:, :, w0:512],
                                             func=ActF.Exp,
                                             scale=1.0 / math.sqrt(DK))
                        if t >= 0:  # mask the 128-wide diagonal sub-block
                            with tc.high_priority():
                                nc.gpsimd.tensor_tensor(ex[:, :, w0:w0 + 128],
                                                        ex[:, :, w0:w0 + 128],
                                                        m2, op=Mult)
                        if prev is not None:
                            pi, pex, pw0 = prev
                            pqi, pti = pi // 4, pi % 4
                            nc.tensor.matmul(ps_oA[:, pw0:512],
                                             vaug[pqi][:, pti, 2 * hp, :],
                                             pex[:, 0, pw0:512],
                                             start=(pi == 0), stop=False,
                                             skip_group_check=True)
                            nc.tensor.matmul(ps_oB[:, pw0:512],
                                             vaug[pqi][:, pti, 2 * hp + 1, :],
                                             pex[:, 1, pw0:512],
                                             start=(pi == 0), stop=False,
                                             skip_group_check=True)
                        prev = (i, ex, w0)
                    pi, pex, pw0 = prev
                    pqi, pti = pi // 4, pi % 4
                    nc.tensor.matmul(ps_oA[:, pw0:512],
                                     vaug[pqi][:, pti, 2 * hp, :],
                                     pex[:, 0, pw0:512],
                                     start=(pi == 0), stop=True,
                                     skip_group_check=True)
                    nc.tensor.matmul(ps_oB[:, pw0:512],
                                     vaug[pqi][:, pti, 2 * hp + 1, :],
                                     pex[:, 1, pw0:512],
                                     start=(pi == 0), stop=True,
                                     skip_group_check=True)
                    # drain: denominator rows + raw (unnormalized) outputs
                    with tc.high_priority():
                        for par, ps_o in ((0, ps_oA), (1, ps_oB)):
                            h = 2 * hp + par
                            a, b = h % 4, h // 4
                            nc.vector.tensor_copy(
                                out=denq[32 * a:32 * a + 1,
                                         b * 512:(b + 1) * 512],
                                in_=ps_o[DK:DK + 1, :])
                        nc.vector.tensor_copy(out=attn[0:64, hp, :],
                                              in_=ps_oA[0:DK, :])
                        nc.vector.tensor_copy(out=attn[64:128, hp, :],
                                              in_=ps_oB[0:DK, :])

                # ---------------- normalize (batched per quarter) ---------
                with nc.allow_low_precision(reason="softmax reciprocal"):
                    nc.vector.reciprocal_approx_fast(out=denq[:], in_=denq[:])
                drec = drp.tile([1, 2 * HPC * 512], f32, tag="drec", name="drec")
                # gather the 8 [1,512] head rows to head-major flat DRAM
                dsrc = denq[:]
                src_g = bass.AP(tensor=dsrc.tensor, offset=dsrc.offset,
                                ap=[[32 * 1024, 4], [512, 2], [1, 512]])
                dr = drec[0, :]
                dr_g = bass.AP(tensor=dr.tensor, offset=dr.offset,
                               ap=[[512, 4], [2048, 2], [1, 512]])
                nc.sync.dma_start(out=dr_g, in_=src_g)
                recq = p1.tile([128, 4, 512], f32, tag="recq", name="recq")
                for par in range(2):
                    # head = 2*hp + par; recq[64*par + p, hp, c] = rec[head, c]
                    src = drec[0, par * 512:par * 512 + 512]
                    src_b = bass.AP(tensor=src.tensor, offset=src.offset,
                                    ap=[[0, 64], [1024, 4], [1, 512]])
                    nc.sync.dma_start(out=recq[par * 64:(par + 1) * 64, :, :],
                                      in_=src_b)
                nc.vector.tensor_tensor(attn[:], attn[:], recq[:], op=Mult)
                pending_oproj.append((q, attn))

            emit_oproj(*pending_oproj.pop())

    nc.compile()
    return nc


def _host_inputs(x, Wq, Wk, Wv, Wo, token_positions):
    import ml_dtypes
    x = np.asarray(x, dtype=np.float32)
    Wq = np.asarray(Wq, dtype=np.float32)
    Wk = np.asarray(Wk, dtype=np.float32)
    Wv = np.asarray(Wv, dtype=np.float32)
    Wo = np.asarray(Wo, dtype=np.float32)
    pos = np.asarray(token_positions, dtype=np.float32)

    half = DK // 2
    inv_freq = THETA ** (-(np.arange(half, dtype=np.float32) * 2.0) / DK)  # [32]
    ang = pos[None, :] * inv_freq[:, None]                                  # [32, S]
    cos32 = np.cos(ang).astype(np.float32)
    sin32 = np.sin(ang).astype(np.float32)
    cosT = np.tile(cos32, (4, 1))                                           # [128, S]
    sinT = np.concatenate([-sin32, sin32, -sin32, sin32], axis=0).astype(np.float32)

    # causal mask for the 128-wide diagonal sub-block: mask[p, c] = 1 if p <= c
    mask128 = (np.arange(128)[:, None] <= np.arange(128)[None, :])
    mask128 = mask128.astype(ml_dtypes.bfloat16)

    perm = np.concatenate([np.arange(0, DK, 2), np.arange(1, DK, 2)])       # evens|odds
    perm_all = (np.arange(HPC)[:, None] * DK + perm[None, :]).reshape(-1)   # [512]

    in_maps = []
    for c in range(N_CORES):
        b, g = c // 2, c % 2
        rows = slice(g * HD, (g + 1) * HD)
        wqT = np.ascontiguousarray(Wq[rows].T)[:, perm_all]                 # [1024, 512]
        wkT = np.ascontiguousarray(Wk[rows].T)[:, perm_all]
        wqkT = np.ascontiguousarray(np.concatenate([wqT, wkT], axis=1))     # [1024, 1024]
        wvT = np.ascontiguousarray(Wv[rows].T)                              # [1024, 512]
        woT = np.ascontiguousarray(Wo[:, rows].T).astype(ml_dtypes.bfloat16)
        in_maps.append({
            "xT": np.ascontiguousarray(x[b].T),
            "wqkT": wqkT,
            "wvT": wvT,
            "woT": woT,
            "cosT": cosT,
            "sinT": sinT,
            "mask128": mask128,
        })
    return in_maps


def run(inputs, trace=False, tmpdir=None):
    """Build (cached), run on 8 cores, return (output, BassKernelResults)."""
    if "nc" not in _prog_cache:
        _prog_cache["nc"] = _build_program()
    nc = _prog_cache["nc"]
    in_maps = _host_inputs(inputs["x"], inputs["Wq"], inputs["Wk"],
                           inputs["Wv"], inputs["Wo"], inputs["token_positions"])
    kw = {}
    if tmpdir is not None:
        kw["tmpdir"] = tmpdir
    res = run_bass_kernel_spmd(nc, in_maps, core_ids=list(range(N_CORES)),
                               trace=trace, **kw)
    out = np.empty((B, S, D), dtype=np.float32)
    for b in range(B):
        acc = res.results[2 * b]["outT"] + res.results[2 * b + 1]["outT"]
        out[b] = acc.T
    return out, res


def kernel(**inputs) -> np.ndarray:
    out, _ = run(inputs, trace=False)
    return out


# revision 21
# speedup vs baseline: 1.2131x; 1.2131x over previous
"""Causal multi-head attention with RoPE on 8 Trainium2 NeuronCores.

Sharding: core c -> batch b = c//2, head-group g = c%2 (8 of 16 heads).
Each core computes q/k/v projections for its batch+heads (fp32r matmuls),
applies RoPE (evens/odds row-permuted layout so the pair-rotation becomes
a 32-partition-group swap done by SBUF-SBUF DMA), runs flash-style causal
attention with transposed scores (softmax sum via an appended ones column
of V -> denominator row in the AV PSUM tile), and a partial output
projection over its head group. Host sums the two per-batch partials.

v2: single interleaved phase structure (projections of quarter q+1 and
output projection of block j-1 provide ready PE work while attention j's
exp chain runs on the Act engine), per-quarter k/v tiles (no false
cross-quarter deps), diagonal score/exp/AV tiles trimmed to the causal
width, exp+psqk drains on Act / everything else elementwise on DVE,
V and attention weights in bf16, and a per-quarter batched softmax
normalization (denominators collected to [8,512], one reciprocal, one
DRAM-bounce broadcast, one in-place multiply).
"""
import math
import sys

sys.path.insert(0, "/opt/trn_rl_repo")

import numpy as np

import concourse.tile as tile
from concourse import bacc, bass, mybir
from concourse.bass_utils import run_bass_kernel_spmd

NUM_HEADS = 16
B, S, D = 4, 2048, 1024
HPC = 8            # heads per core
DK = 64
HD = HPC * DK      # 512 head dims per core
THETA = 10000.0
N_CORES = 8
KC = D // 128      # 8 contraction chunks for projections
NQ = 4             # s-quarters of 512

f32 = mybir.dt.float32
f32r = mybir.dt.float32r
bf16 = mybir.dt.bfloat16
ActF = mybir.ActivationFunctionType
Mult = mybir.AluOpType.mult

_prog_cache = {}


def _build_program():
    nc = bacc.Bacc("TRN2", target_bir_lowering=False, debug=False,
                   enable_asserts=False, num_devices=N_CORES)
    xT_d = nc.dram_tensor("xT", [D, S], f32r, kind="ExternalInput").ap()
    wqk_d = nc.dram_tensor("wqkT", [D, 2 * HD], f32r, kind="ExternalInput").ap()
    wv_d = nc.dram_tensor("wvT", [D, HD], f32r, kind="ExternalInput").ap()
    wo_d = nc.dram_tensor("woT", [HD, D], bf16, kind="ExternalInput").ap()
    cos_d = nc.dram_tensor("cosT", [128, S], f32, kind="ExternalInput").ap()
    sin_d = nc.dram_tensor("sinT", [128, S], f32, kind="ExternalInput").ap()
    mask_d = nc.dram_tensor("mask128", [128, 128], bf16, kind="ExternalInput").ap()
    out_d = nc.dram_tensor("outT", [D, S], f32, kind="ExternalOutput").ap()

    with tile.TileContext(nc) as tc:
        with tc.tile_pool(name="p1", bufs=1) as p1, \
             tc.tile_pool(name="p2", bufs=2) as p2, \
             tc.tile_pool(name="pss", bufs=2, space="PSUM") as pss, \
             tc.tile_pool(name="pso", bufs=1, space="PSUM") as pso_pool, \
             tc.tile_pool(name="ppr", bufs=2, space="PSUM") as ppr, \
             tc.tile_pool(name="drp", bufs=2, space="DRAM") as drp:

            # ---------------- persistent SBUF ----------------------------
            wv_sb = p1.tile([128, KC, HD], f32r, tag="wv")
            wqk_sb = p1.tile([128, KC, 2 * HD], f32r, tag="wqk")
            wo_sb = p1.tile([128, HD // 128, D], bf16, tag="wo")
            mask_sb = p1.tile([128, 128], bf16, tag="mask")
            krot = [p1.tile([128, 4, 512], f32r, tag=f"krot{q}", name=f"krot{q}")
                    for q in range(NQ)]
            vaug = [p1.tile([128, 4, HPC, DK + 1], bf16, tag=f"vaug{q}",
                            name=f"vaug{q}") for q in range(NQ)]

            # weight/table loads: wv+cos/sin+mask+wo on sync HWDGE,
            # wqk on the scalar HWDGE ring (Act idle at startup)
            for k in range(KC):
                nc.sync.dma_start(out=wv_sb[:, k, :],
                                  in_=wv_d[k * 128:(k + 1) * 128, :])
            nc.scalar.dma_start(out=wqk_sb[:],
                                in_=wqk_d.rearrange("(k p) n -> p k n", p=128))
            nc.sync.dma_start(out=mask_sb[:], in_=mask_d[:])
            for q in range(NQ):
                nc.vector.memset(vaug[q][:, :, :, DK], 1.0)
            denq = p1.tile([128, 1024], f32, tag="denq", name="denq")
            nc.vector.memset(denq[:], 1.0)

            pending_oproj = []

            def emit_oproj(jq, attn_t):
                # output projection for block jq: emitted after quarter
                # jq+1's projections so the shared 'pp' PSUM ring grants
                # slots in a feasible execution order
                slj = slice(jq * 512, (jq + 1) * 512)
                for mo in range(D // 128):
                    pso = ppr.tile([128, 512], f32, tag="pp", name="pso")
                    for kc2 in range(HD // 128):
                        nc.tensor.matmul(pso[:],
                                         wo_sb[:, kc2, mo * 128:(mo + 1) * 128],
                                         attn_t[:, kc2, :],
                                         start=(kc2 == 0),
                                         stop=(kc2 == HD // 128 - 1))
                    ob = p2.tile([128, 512], f32, tag="ob", name="ob")
                    nc.vector.tensor_copy(out=ob[:], in_=pso[:])
                    nc.sync.dma_start(out=out_d[mo * 128:(mo + 1) * 128, slj],
                                      in_=ob[:])

            for q in range(NQ):
                sl = slice(q * 512, (q + 1) * 512)

                # ---------------- projections for quarter q --------------
                cosq = p2.tile([128, 512], f32, tag="cosq", name="cosq")
                nc.sync.dma_start(out=cosq[:], in_=cos_d[:, sl])
                sinq = p2.tile([128, 512], f32, tag="sinq", name="sinq")
                nc.sync.dma_start(out=sinq[:], in_=sin_d[:, sl])
                qrot = p2.tile([128, 4, 512], f32r, tag="qrot", name="qrot")

                xq = p2.tile([128, KC, 512], f32r, tag="xq", name="xq")
                for kh in range(4):
                    nc.gpsimd.dma_start(
                        out=xq[:, 2 * kh:2 * kh + 2, :],
                        in_=xT_d[kh * 256:(kh + 1) * 256, sl].rearrange(
                            "(k p) c -> p k c", p=128))

                # V projection: 4 s-tiles of 128
                for m in range(4):
                    psv = ppr.tile([128, 512], f32, tag="pp", name="psv")
                    for k in range(KC):
                        nc.tensor.matmul(psv[:],
                                         xq[:, k, m * 128:(m + 1) * 128],
                                         wv_sb[:, k, :],
                                         start=(k == 0), stop=(k == KC - 1))
                    nc.vector.tensor_copy(
                        out=vaug[q][:, m, :, 0:DK],
                        in_=psv[:].rearrange("p (h d) -> p h d", h=HPC))

                # QK projection + RoPE: 8 output row-chunks
                for m in range(2 * KC // 2):
                    xy = p2.tile([128, 2, 512], f32, tag="xy", name="xy")
                    pq = ppr.tile([128, 512], f32, tag="pp", name="psqk")
                    for k in range(KC):
                        nc.tensor.matmul(pq[:],
                                         wqk_sb[:, k, m * 128:(m + 1) * 128],
                                         xq[:, k, :],
                                         start=(k == 0), stop=(k == KC - 1))
                    nc.scalar.copy(out=xy[:, 0, :], in_=pq[:])
                    for gq in range(4):
                        a, bb = 32 * gq, 32 * (gq ^ 1)
                        nc.gpsimd.dma_start(out=xy[a:a + 32, 1, :],
                                            in_=xy[bb:bb + 32, 0, :])
                    t1 = p2.tile([128, 512], f32, tag="t1", name="t1")
                    nc.vector.tensor_mul(t1[:], xy[:, 0, :], cosq[:])
                    nc.vector.tensor_mul(xy[:, 1, :], xy[:, 1, :], sinq[:])
                    dest = qrot[:, m, :] if m < 4 else krot[q][:, m - 4, :]
                    nc.vector.tensor_add(dest, t1[:], xy[:, 1, :])

                if q == 1:
                    nc.sync.dma_start(
                        out=wo_sb[:],
                        in_=wo_d.rearrange("(k p) m -> p k m", p=128))
                if pending_oproj:
                    emit_oproj(*pending_oproj.pop())

                # ---------------- attention for block j=q -----------------
                # denominators staged at partition starts {0,32,64,96} x 2
                # column blocks: head h -> [32*(h%4), (h//4)*512]
                attn = p2.tile([128, 4, 512], bf16, tag="attn", name="attn")
                n_i = 4 * q + 4
                m2 = bass.AP(tensor=mask_sb[:].tensor, offset=mask_sb[:].offset,
                             ap=[mask_sb[:].ap[0], [0, 2], mask_sb[:].ap[1]])
                for hp in range(4):
                    ps_oA = pso_pool.tile([DK + 1, 512], f32, tag="psoA",
                                          name="ps_oA")
                    ps_oB = pso_pool.tile([DK + 1, 512], f32, tag="psoB",
                                          name="ps_oB")
                    prev = None
                    for i in range(n_i):
                        t = i - 4 * q
                        w0 = 128 * t if t > 0 else 0
                        qi, ti = i // 4, i % 4
                        ps_s = pss.tile([128, 2, 512], f32, tag="pss",
                                        name="ps_s")
                        nc.tensor.matmul(
                            ps_s[:, 0, w0:512],
                            krot[qi][0:64, hp, ti * 128:(ti + 1) * 128],
                            qrot[0:64, hp, w0:512], start=True, stop=True)
                        nc.tensor.matmul(
                            ps_s[:, 1, w0:512],
                            krot[qi][64:128, hp, ti * 128:(ti + 1) * 128],
                            qrot[64:128, hp, w0:512], start=True, stop=True)
                        ex = p2.tile([128, 2, 512], bf16, tag="ex", name="ex")
                        nc.scalar.activation(out=ex[:, :, w0:512],
                                             in_=ps_s[:, :, w0:512],
                                             func=ActF.Exp,
                                             scale=1.0 / math.sqrt(DK))
                        if t >= 0:  # mask the 128-wide diagonal sub-block
                            with tc.high_priority():
                                nc.vector.tensor_tensor(ex[:, :, w0:w0 + 128],
                                                        ex[:, :, w0:w0 + 128],
                                                        m2, op=Mult)
                        if prev is not None:
                            pi, pex, pw0 = prev
                            pqi, pti = pi // 4, pi % 4
                            nc.tensor.matmul(ps_oA[:, pw0:512],
                                             vaug[pqi][:, pti, 2 * hp, :],
                                             pex[:, 0, pw0:512],
                                             start=(pi == 0), stop=False,
                                             skip_group_check=True)
                            nc.tensor.matmul(ps_oB[:, pw0:512],
                                             vaug[pqi][:, pti, 2 * hp + 1, :],
                                             pex[:, 1, pw0:512],
                                             start=(pi == 0), stop=False,
                                             skip_group_check=True)
                        prev = (i, ex, w0)
                    pi, pex, pw0 = prev
                    pqi, pti = pi // 4, pi % 4
                    nc.tensor.matmul(ps_oA[:, pw0:512],
                                     vaug[pqi][:, pti, 2 * hp, :],
                                     pex[:, 0, pw0:512],
                                     start=(pi == 0), stop=True,
                                     skip_group_check=True)
                    nc.tensor.matmul(ps_oB[:, pw0:512],
                                     vaug[pqi][:, pti, 2 * hp + 1, :],
                                     pex[:, 1, pw0:512],
                                     start=(pi == 0), stop=True,
                                     skip_group_check=True)
                    # drain: denominator rows + raw (unnormalized) outputs
                    with tc.high_priority():
                        for par, ps_o in ((0, ps_oA), (1, ps_oB)):
                            h = 2 * hp + par
                            a, b = h % 4, h // 4
                            nc.vector.tensor_copy(
                                out=denq[32 * a:32 * a + 1,
                                         b * 512:(b + 1) * 512],
                                in_=ps_o[DK:DK + 1, :])
                        nc.vector.tensor_copy(out=attn[0:64, hp, :],
                                              in_=ps_oA[0:DK, :])
                        nc.vector.tensor_copy(out=attn[64:128, hp, :],
                                              in_=ps_oB[0:DK, :])

                # ---------------- normalize (batched per quarter) ---------
                with nc.allow_low_precision(reason="softmax reciprocal"):
                    nc.vector.reciprocal_approx_fast(out=denq[:], in_=denq[:])
                drec = drp.tile([1, 2 * HPC * 512], f32, tag="drec", name="drec")
                # gather the 8 [1,512] head rows to head-major flat DRAM
                dsrc = denq[:]
                src_g = bass.AP(tensor=dsrc.tensor, offset=dsrc.offset,
                                ap=[[32 * 1024, 4], [512, 2], [1, 512]])
                dr = drec[0, :]
                dr_g = bass.AP(tensor=dr.tensor, offset=dr.offset,
                               ap=[[512, 4], [2048, 2], [1, 512]])
                nc.sync.dma_start(out=dr_g, in_=src_g)
                recq = p1.tile([128, 4, 512], f32, tag="recq", name="recq")
                for par in range(2):
                    # head = 2*hp + par; recq[64*par + p, hp, c] = rec[head, c]
                    src = drec[0, par * 512:par * 512 + 512]
                    src_b = bass.AP(tensor=src.tensor, offset=src.offset,
                                    ap=[[0, 64], [1024, 4], [1, 512]])
                    nc.sync.dma_start(out=recq[par * 64:(par + 1) * 64, :, :],
                                      in_=src_b)
                nc.vector.tensor_tensor(attn[:], attn[:], recq[:], op=Mult)
                pending_oproj.append((q, attn))

            emit_oproj(*pending_oproj.pop())

    nc.compile()
    return nc


def _host_inputs(x, Wq, Wk, Wv, Wo, token_positions):
    import ml_dtypes
    x = np.asarray(x, dtype=np.float32)
    Wq = np.asarray(Wq, dtype=np.float32)
    Wk = np.asarray(Wk, dtype=np.float32)
    Wv = np.asarray(Wv, dtype=np.float32)
    Wo = np.asarray(Wo, dtype=np.float32)
    pos = np.asarray(token_positions, dtype=np.float32)

    half = DK // 2
    inv_freq = THETA ** (-(np.arange(half, dtype=np.float32) * 2.0) / DK)  # [32]
    ang = pos[None, :] * inv_freq[:, None]                                  # [32, S]
    cos32 = np.cos(ang).astype(np.float32)
    sin32 = np.sin(ang).astype(np.float32)
    cosT = np.tile(cos32, (4, 1))                                           # [128, S]
    sinT = np.concatenate([-sin32, sin32, -sin32, sin32], axis=0).astype(np.float32)

    # causal mask for the 128-wide diagonal sub-block: mask[p, c] = 1 if p <= c
    mask128 = (np.arange(128)[:, None] <= np.arange(128)[None, :])
    mask128 = mask128.astype(ml_dtypes.bfloat16)

    perm = np.concatenate([np.arange(0, DK, 2), np.arange(1, DK, 2)])       # evens|odds
    perm_all = (np.arange(HPC)[:, None] * DK + perm[None, :]).reshape(-1)   # [512]

    in_maps = []
    for c in range(N_CORES):
        b, g = c // 2, c % 2
        rows = slice(g * HD, (g + 1) * HD)
        wqT = np.ascontiguousarray(Wq[rows].T)[:, perm_all]                 # [1024, 512]
        wkT = np.ascontiguousarray(Wk[rows].T)[:, perm_all]
        wqkT = np.ascontiguousarray(np.concatenate([wqT, wkT], axis=1))     # [1024, 1024]
        wvT = np.ascontiguousarray(Wv[rows].T)                              # [1024, 512]
        woT = np.ascontiguousarray(Wo[:, rows].T).astype(ml_dtypes.bfloat16)
        in_maps.append({
            "xT": np.ascontiguousarray(x[b].T),
            "wqkT": wqkT,
            "wvT": wvT,
            "woT": woT,
            "cosT": cosT,
            "sinT": sinT,
            "mask128": mask128,
        })
    return in_maps


def run(inputs, trace=False, tmpdir=None):
    """Build (cached), run on 8 cores, return (output, BassKernelResults)."""
    if "nc" not in _prog_cache:
        _prog_cache["nc"] = _build_program()
    nc = _prog_cache["nc"]
    in_maps = _host_inputs(inputs["x"], inputs["Wq"], inputs["Wk"],
                           inputs["Wv"], inputs["Wo"], inputs["token_positions"])
    kw = {}
    if tmpdir is not None:
        kw["tmpdir"] = tmpdir
    res = run_bass_kernel_spmd(nc, in_maps, core_ids=list(range(N_CORES)),
                               trace=trace, **kw)
    out = np.empty((B, S, D), dtype=np.float32)
    for b in range(B):
        acc = res.results[2 * b]["outT"] + res.results[2 * b + 1]["outT"]
        out[b] = acc.T
    return out, res


def kernel(**inputs) -> np.ndarray:
    out, _ = run(inputs, trace=False)
    return out


# revision 23
# speedup vs baseline: 1.2754x; 1.0514x over previous
"""Causal multi-head attention with RoPE on 8 Trainium2 NeuronCores.

Sharding: core c -> batch b = c//2, head-group g = c%2 (8 of 16 heads).
Each core computes q/k/v projections for its batch+heads (fp32r matmuls),
applies RoPE (evens/odds row-permuted layout so the pair-rotation becomes
a 32-partition-group swap done by SBUF-SBUF DMA), runs flash-style causal
attention with transposed scores (softmax sum via an appended ones column
of V -> denominator row in the AV PSUM tile), and a partial output
projection over its head group. Host sums the two per-batch partials.

v2: single interleaved phase structure (projections of quarter q+1 and
output projection of block j-1 provide ready PE work while attention j's
exp chain runs on the Act engine), per-quarter k/v tiles (no false
cross-quarter deps), diagonal score/exp/AV tiles trimmed to the causal
width, exp+psqk drains on Act / everything else elementwise on DVE,
V and attention weights in bf16, and a per-quarter batched softmax
normalization (denominators collected to [8,512], one reciprocal, one
DRAM-bounce broadcast, one in-place multiply).
"""
import math
import sys

sys.path.insert(0, "/opt/trn_rl_repo")

import numpy as np

import concourse.tile as tile
from concourse import bacc, bass, mybir
from concourse.bass_utils import run_bass_kernel_spmd

NUM_HEADS = 16
B, S, D = 4, 2048, 1024
HPC = 8            # heads per core
DK = 64
HD = HPC * DK      # 512 head dims per core
THETA = 10000.0
N_CORES = 8
KC = D // 128      # 8 contraction chunks for projections
NQ = 4             # s-quarters of 512

f32 = mybir.dt.float32
f32r = mybir.dt.float32r
bf16 = mybir.dt.bfloat16
ActF = mybir.ActivationFunctionType
Mult = mybir.AluOpType.mult

_prog_cache = {}


def _build_program():
    nc = bacc.Bacc("TRN2", target_bir_lowering=False, debug=False,
                   enable_asserts=False, num_devices=N_CORES)
    xT_d = nc.dram_tensor("xT", [D, S], f32r, kind="ExternalInput").ap()
    wqk_d = nc.dram_tensor("wqkT", [D, 2 * HD], f32r, kind="ExternalInput").ap()
    wv_d = nc.dram_tensor("wvT", [D, HD], f32r, kind="ExternalInput").ap()
    wo_d = nc.dram_tensor("woT", [HD, D], bf16, kind="ExternalInput").ap()
    cos_d = nc.dram_tensor("cosT", [128, S], bf16, kind="ExternalInput").ap()
    sin_d = nc.dram_tensor("sinT", [128, S], bf16, kind="ExternalInput").ap()
    mask_d = nc.dram_tensor("mask128", [128, 128], bf16, kind="ExternalInput").ap()
    out_d = nc.dram_tensor("outT", [D, S], f32, kind="ExternalOutput").ap()

    with tile.TileContext(nc) as tc:
        with tc.tile_pool(name="p1", bufs=1) as p1, \
             tc.tile_pool(name="p2", bufs=2) as p2, \
             tc.tile_pool(name="p3", bufs=3) as p3, \
             tc.tile_pool(name="pss", bufs=2, space="PSUM") as pss, \
             tc.tile_pool(name="pso", bufs=1, space="PSUM") as pso_pool, \
             tc.tile_pool(name="ppr", bufs=2, space="PSUM") as ppr, \
             tc.tile_pool(name="drp", bufs=2, space="DRAM") as drp:

            # ---------------- persistent SBUF ----------------------------
            wv_sb = p1.tile([128, KC, HD], f32r, tag="wv")
            wqk_sb = p1.tile([128, KC, 2 * HD], f32r, tag="wqk")
            wo_sb = p1.tile([128, HD // 128, D], bf16, tag="wo")
            mask_sb = p1.tile([128, 128], bf16, tag="mask")
            krot = [p1.tile([128, 4, 512], bf16, tag=f"krot{q}", name=f"krot{q}")
                    for q in range(NQ)]
            vaug = [p1.tile([128, 4, HPC, DK + 1], bf16, tag=f"vaug{q}",
                            name=f"vaug{q}") for q in range(NQ)]

            # weight/table loads: wv+cos/sin+mask+wo on sync HWDGE,
            # wqk on the scalar HWDGE ring (Act idle at startup)
            for k in range(KC):
                nc.sync.dma_start(out=wv_sb[:, k, :],
                                  in_=wv_d[k * 128:(k + 1) * 128, :])
            nc.scalar.dma_start(out=wqk_sb[:],
                                in_=wqk_d.rearrange("(k p) n -> p k n", p=128))
            nc.sync.dma_start(out=mask_sb[:], in_=mask_d[:])
            for q in range(NQ):
                nc.vector.memset(vaug[q][:, :, :, DK], 1.0)
            denq = p1.tile([128, 1024], f32, tag="denq", name="denq")
            nc.vector.memset(denq[:], 1.0)

            pending_oproj = []

            def emit_oproj(jq, attn_t):
                # output projection for block jq: emitted after quarter
                # jq+1's projections so the shared 'pp' PSUM ring grants
                # slots in a feasible execution order
                slj = slice(jq * 512, (jq + 1) * 512)
                for mo in range(D // 128):
                    pso = ppr.tile([128, 512], f32, tag="pp", name="pso")
                    for kc2 in range(HD // 128):
                        nc.tensor.matmul(pso[:],
                                         wo_sb[:, kc2, mo * 128:(mo + 1) * 128],
                                         attn_t[:, kc2, :],
                                         start=(kc2 == 0),
                                         stop=(kc2 == HD // 128 - 1))
                    ob = p2.tile([128, 512], f32, tag="ob", name="ob")
                    nc.vector.tensor_copy(out=ob[:], in_=pso[:])
                    nc.sync.dma_start(out=out_d[mo * 128:(mo + 1) * 128, slj],
                                      in_=ob[:])

            for q in range(NQ):
                sl = slice(q * 512, (q + 1) * 512)

                # ---------------- projections for quarter q --------------
                cosq = p2.tile([128, 512], bf16, tag="cosq", name="cosq")
                nc.sync.dma_start(out=cosq[:], in_=cos_d[:, sl])
                sinq = p2.tile([128, 512], bf16, tag="sinq", name="sinq")
                nc.sync.dma_start(out=sinq[:], in_=sin_d[:, sl])
                qrot = p2.tile([128, 4, 512], bf16, tag="qrot", name="qrot")

                xq = p3.tile([128, KC, 512], f32r, tag="xq", name="xq")
                for kh in range(4):
                    nc.gpsimd.dma_start(
                        out=xq[:, 2 * kh:2 * kh + 2, :],
                        in_=xT_d[kh * 256:(kh + 1) * 256, sl].rearrange(
                            "(k p) c -> p k c", p=128))

                # V projection: 4 s-tiles of 128
                for m in range(4):
                    psv = ppr.tile([128, 512], f32, tag="pp", name="psv")
                    for k in range(KC):
                        nc.tensor.matmul(psv[:],
                                         xq[:, k, m * 128:(m + 1) * 128],
                                         wv_sb[:, k, :],
                                         start=(k == 0), stop=(k == KC - 1))
                    nc.vector.tensor_copy(
                        out=vaug[q][:, m, :, 0:DK],
                        in_=psv[:].rearrange("p (h d) -> p h d", h=HPC))

                # QK projection + RoPE: 8 output row-chunks
                for m in range(2 * KC // 2):
                    xy = p3.tile([128, 2, 512], bf16, tag="xy", name="xy")
                    pq = ppr.tile([128, 512], f32, tag="pp", name="psqk")
                    for k in range(KC):
                        nc.tensor.matmul(pq[:],
                                         wqk_sb[:, k, m * 128:(m + 1) * 128],
                                         xq[:, k, :],
                                         start=(k == 0), stop=(k == KC - 1))
                    nc.scalar.copy(out=xy[:, 0, :], in_=pq[:])
                    for gq in range(4):
                        a, bb = 32 * gq, 32 * (gq ^ 1)
                        nc.gpsimd.dma_start(out=xy[a:a + 32, 1, :],
                                            in_=xy[bb:bb + 32, 0, :])
                    t1 = p3.tile([128, 512], bf16, tag="t1", name="t1")
                    nc.vector.tensor_mul(t1[:], xy[:, 0, :], cosq[:])
                    nc.vector.tensor_mul(xy[:, 1, :], xy[:, 1, :], sinq[:])
                    dest = qrot[:, m, :] if m < 4 else krot[q][:, m - 4, :]
                    nc.vector.tensor_add(dest, t1[:], xy[:, 1, :])

                if q == 1:
                    nc.sync.dma_start(
                        out=wo_sb[:],
                        in_=wo_d.rearrange("(k p) m -> p k m", p=128))
                if pending_oproj:
                    emit_oproj(*pending_oproj.pop())

                # ---------------- attention for block j=q -----------------
                # denominators staged at partition starts {0,32,64,96} x 2
                # column blocks: head h -> [32*(h%4), (h//4)*512]
                attn = p2.tile([128, 4, 512], bf16, tag="attn", name="attn")
                n_i = 4 * q + 4
                m2 = bass.AP(tensor=mask_sb[:].tensor, offset=mask_sb[:].offset,
                             ap=[mask_sb[:].ap[0], [0, 2], mask_sb[:].ap[1]])
                for hp in range(4):
                    ps_oA = pso_pool.tile([DK + 1, 512], f32, tag="psoA",
                                          name="ps_oA")
                    ps_oB = pso_pool.tile([DK + 1, 512], f32, tag="psoB",
                                          name="ps_oB")
                    prev = None
                    for i in range(n_i):
                        t = i - 4 * q
                        w0 = 128 * t if t > 0 else 0
                        qi, ti = i // 4, i % 4
                        ps_s = pss.tile([128, 2, 512], f32, tag="pss",
                                        name="ps_s")
                        nc.tensor.matmul(
                            ps_s[:, 0, w0:512],
                            krot[qi][0:64, hp, ti * 128:(ti + 1) * 128],
                            qrot[0:64, hp, w0:512], start=True, stop=True)
                        nc.tensor.matmul(
                            ps_s[:, 1, w0:512],
                            krot[qi][64:128, hp, ti * 128:(ti + 1) * 128],
                            qrot[64:128, hp, w0:512], start=True, stop=True)
                        ex = p3.tile([128, 2, 512], bf16, tag="ex", name="ex")
                        nc.scalar.activation(out=ex[:, :, w0:512],
                                             in_=ps_s[:, :, w0:512],
                                             func=ActF.Exp,
                                             scale=1.0 / math.sqrt(DK))
                        if t >= 0:  # mask the 128-wide diagonal sub-block
                            with tc.high_priority():
                                nc.vector.tensor_tensor(ex[:, :, w0:w0 + 128],
                                                        ex[:, :, w0:w0 + 128],
                                                        m2, op=Mult)
                        if prev is not None:
                            pi, pex, pw0 = prev
                            pqi, pti = pi // 4, pi % 4
                            nc.tensor.matmul(ps_oA[:, pw0:512],
                                             vaug[pqi][:, pti, 2 * hp, :],
                                             pex[:, 0, pw0:512],
                                             start=(pi == 0), stop=False,
                                             skip_group_check=True)
                            nc.tensor.matmul(ps_oB[:, pw0:512],
                                             vaug[pqi][:, pti, 2 * hp + 1, :],
                                             pex[:, 1, pw0:512],
                                             start=(pi == 0), stop=False,
                                             skip_group_check=True)
                        prev = (i, ex, w0)
                    pi, pex, pw0 = prev
                    pqi, pti = pi // 4, pi % 4
                    nc.tensor.matmul(ps_oA[:, pw0:512],
                                     vaug[pqi][:, pti, 2 * hp, :],
                                     pex[:, 0, pw0:512],
                                     start=(pi == 0), stop=True,
                                     skip_group_check=True)
                    nc.tensor.matmul(ps_oB[:, pw0:512],
                                     vaug[pqi][:, pti, 2 * hp + 1, :],
                                     pex[:, 1, pw0:512],
                                     start=(pi == 0), stop=True,
                                     skip_group_check=True)
                    # drain: denominator rows + raw (unnormalized) outputs
                    with tc.high_priority():
                        for par, ps_o in ((0, ps_oA), (1, ps_oB)):
                            h = 2 * hp + par
                            a, b = h % 4, h // 4
                            nc.vector.tensor_copy(
                                out=denq[32 * a:32 * a + 1,
                                         b * 512:(b + 1) * 512],
                                in_=ps_o[DK:DK + 1, :])
                        nc.vector.tensor_copy(out=attn[0:64, hp, :],
                                              in_=ps_oA[0:DK, :])
                        nc.vector.tensor_copy(out=attn[64:128, hp, :],
                                              in_=ps_oB[0:DK, :])

                # ---------------- normalize (batched per quarter) ---------
                with nc.allow_low_precision(reason="softmax reciprocal"):
                    nc.vector.reciprocal_approx_fast(out=denq[:], in_=denq[:])
                drec = drp.tile([1, 2 * HPC * 512], f32, tag="drec", name="drec")
                # gather the 8 [1,512] head rows to head-major flat DRAM
                dsrc = denq[:]
                src_g = bass.AP(tensor=dsrc.tensor, offset=dsrc.offset,
                                ap=[[32 * 1024, 4], [512, 2], [1, 512]])
                dr = drec[0, :]
                dr_g = bass.AP(tensor=dr.tensor, offset=dr.offset,
                               ap=[[512, 4], [2048, 2], [1, 512]])
                nc.sync.dma_start(out=dr_g, in_=src_g)
                recq = p1.tile([128, 4, 512], f32, tag="recq", name="recq")
                for par in range(2):
                    # head = 2*hp + par; recq[64*par + p, hp, c] = rec[head, c]
                    src = drec[0, par * 512:par * 512 + 512]
                    src_b = bass.AP(tensor=src.tensor, offset=src.offset,
                                    ap=[[0, 64], [1024, 4], [1, 512]])
                    nc.sync.dma_start(out=recq[par * 64:(par + 1) * 64, :, :],
                                      in_=src_b)
                for hp in range(4):
                    nc.vector.tensor_tensor(attn[:, hp, :], attn[:, hp, :],
                                            recq[:, hp, :], op=Mult)
                pending_oproj.append((q, attn))

            emit_oproj(*pending_oproj.pop())

    nc.compile()
    return nc


def _host_inputs(x, Wq, Wk, Wv, Wo, token_positions):
    import ml_dtypes
    x = np.asarray(x, dtype=np.float32)
    Wq = np.asarray(Wq, dtype=np.float32)
    Wk = np.asarray(Wk, dtype=np.float32)
    Wv = np.asarray(Wv, dtype=np.float32)
    Wo = np.asarray(Wo, dtype=np.float32)
    pos = np.asarray(token_positions, dtype=np.float32)

    half = DK // 2
    inv_freq = THETA ** (-(np.arange(half, dtype=np.float32) * 2.0) / DK)  # [32]
    ang = pos[None, :] * inv_freq[:, None]                                  # [32, S]
    cos32 = np.cos(ang).astype(np.float32)
    sin32 = np.sin(ang).astype(np.float32)
    cosT = np.tile(cos32, (4, 1)).astype(ml_dtypes.bfloat16)               # [128, S]
    sinT = np.concatenate([-sin32, sin32, -sin32, sin32],
                          axis=0).astype(ml_dtypes.bfloat16)

    # causal mask for the 128-wide diagonal sub-block: mask[p, c] = 1 if p <= c
    mask128 = (np.arange(128)[:, None] <= np.arange(128)[None, :])
    mask128 = mask128.astype(ml_dtypes.bfloat16)

    perm = np.concatenate([np.arange(0, DK, 2), np.arange(1, DK, 2)])       # evens|odds
    perm_all = (np.arange(HPC)[:, None] * DK + perm[None, :]).reshape(-1)   # [512]

    in_maps = []
    for c in range(N_CORES):
        b, g = c // 2, c % 2
        rows = slice(g * HD, (g + 1) * HD)
        wqT = np.ascontiguousarray(Wq[rows].T)[:, perm_all]                 # [1024, 512]
        wkT = np.ascontiguousarray(Wk[rows].T)[:, perm_all]
        wqkT = np.ascontiguousarray(np.concatenate([wqT, wkT], axis=1))     # [1024, 1024]
        wvT = np.ascontiguousarray(Wv[rows].T)                              # [1024, 512]
        woT = np.ascontiguousarray(Wo[:, rows].T).astype(ml_dtypes.bfloat16)
        in_maps.append({
            "xT": np.ascontiguousarray(x[b].T),
            "wqkT": wqkT,
            "wvT": wvT,
            "woT": woT,
            "cosT": cosT,
            "sinT": sinT,
            "mask128": mask128,
        })
    return in_maps


def run(inputs, trace=False, tmpdir=None):
    """Build (cached), run on 8 cores, return (output, BassKernelResults)."""
    if "nc" not in _prog_cache:
        _prog_cache["nc"] = _build_program()
    nc = _prog_cache["nc"]
    in_maps = _host_inputs(inputs["x"], inputs["Wq"], inputs["Wk"],
                           inputs["Wv"], inputs["Wo"], inputs["token_positions"])
    kw = {}
    if tmpdir is not None:
        kw["tmpdir"] = tmpdir
    res = run_bass_kernel_spmd(nc, in_maps, core_ids=list(range(N_CORES)),
                               trace=trace, **kw)
    out = np.empty((B, S, D), dtype=np.float32)
    for b in range(B):
        acc = res.results[2 * b]["outT"] + res.results[2 * b + 1]["outT"]
        out[b] = acc.T
    return out, res


def kernel(**inputs) -> np.ndarray:
    out, _ = run(inputs, trace=False)
    return out
